# revision 31
# baseline (speedup 1.0000x reference)
"""Trainium2 Bass kernel for nn_Attention (B=4, N=2048, D=1024, H=16, Hd=64).

Sharding: 8 cores = 4 batches x 2 head-groups. Core c handles batch c//2 and
heads [ (c%2)*8, (c%2)*8+8 ).  Each core computes qkv projections for its
heads, attention, and a partial output projection (contraction over its 512
head-dims of W_proj). Host sums the two partials per batch and adds b_proj.

Per-core kernel (all matmuls bf16 with fp32 PSUM accumulation). Work is
organized as 16 attention units (8 heads x 2 query halves of 1024):
  - qkT[f][p, t]: q/k features on partitions (f 0-3 = Q of head pairs,
    4-7 = K), produced by background qk_sub units.
  - v_all[k, h*1040 + kt*65 + j]: V in keys-major per-head 65-wide blocks
    whose 65th column is constant 1.0. The PV matmul (lhsT = V block,
    M=65) then accumulates BOTH U^T (rows 0-63) and the softmax
    denominators sum_k E[k,q] (row 64) in one PSUM tile - no separate
    E-accumulation pass is needed.
  - per unit: 16 key tiles: S^T = K^T Q (keys on partitions), E = exp
    (ScalarE, bf16), U^T/denoms += V_ext^T E (PSUM, lag-2 behind exp).
    Tail: recip(denoms) -> gpsimd partition_broadcast -> normalize into
    uhat (bf16).
  - proj: y[q, e] partial = sum_hd uhat Wp, background-woven; host adds
    the pair partials + bias.
"""

import os
import sys
import types

import numpy as np

# --- environment bootstrap (grading env == dev env: axon-tunneled trn2) ----
for _p in ("/opt/trn_rl_repo", "/root/.axon_site/_ro/trn_rl_repo"):
    if _p not in sys.path and os.path.isdir(_p):
        sys.path.append(_p)

import ml_dtypes  # noqa: E402

BF16 = ml_dtypes.bfloat16


def _install_ntff_shim():
    """antenv.axon_hooks is missing on this image; provide it and register the
    ctypes NTFF hook so trace=True can report HW exec time."""
    if "antenv.axon_hooks" in sys.modules:
        return
    mod = types.ModuleType("antenv.axon_hooks")
    mod._hook = None
    mod.set_axon_ntff_profile_hook = lambda h: setattr(mod, "_hook", h)
    mod.get_axon_ntff_profile_hook = lambda: mod._hook
    sys.modules["antenv.axon_hooks"] = mod
    try:
        import antenv

        antenv.axon_hooks = mod
    except ImportError:
        pass
    try:
        from trn_agent_boot.trn_boot import _ntff_profile_via_ctypes

        hook = _ntff_profile_via_ctypes("/opt/axon/libaxon_pjrt.so")
        if hook is not None:
            mod.set_axon_ntff_profile_hook(hook)
    except Exception:
        pass


_install_ntff_shim()

import concourse.bacc as bacc  # noqa: E402
import concourse.bass as bass  # noqa: E402
import concourse.tile as tile  # noqa: E402
import concourse.bass_isa as bass_isa  # noqa: E402
from concourse import mybir  # noqa: E402
import concourse.bass_utils as bass_utils  # noqa: E402

# no S3 in the container; keep NTFF artifacts local
bass_utils.upload_artifacts = lambda tmpdir: tmpdir

F32 = mybir.dt.float32
BF = mybir.dt.bfloat16
FP16 = mybir.dt.float16
EXP = mybir.ActivationFunctionType.Exp
CPY = mybir.ActivationFunctionType.Copy

N_CORES = 8
NT = 2048  # tokens
D = 1024  # d_model
NH_LOC = 8  # heads per core
HD = 64  # head dim
SCALE = HD**-0.5


DEBUG_DUMP = bool(int(os.environ.get("KBASS_DEBUG", "0")))


def _body(tc: "tile.TileContext", ctx, y, xT, wqk, wv, wp, dbg=None):
    nc = tc.nc

    wpool = ctx.enter_context(tc.tile_pool(name="wpool", bufs=1))
    qkpool = ctx.enter_context(tc.tile_pool(name="qkpool", bufs=1))
    vpool = ctx.enter_context(tc.tile_pool(name="vpool", bufs=1))
    upool = ctx.enter_context(tc.tile_pool(name="upool", bufs=1))
    urawpool = ctx.enter_context(tc.tile_pool(name="urawpool", bufs=2))
    partpool = ctx.enter_context(tc.tile_pool(name="partpool", bufs=16))
    epool = ctx.enter_context(tc.tile_pool(name="epool", bufs=3))
    spool = ctx.enter_context(tc.tile_pool(name="spool", bufs=2))
    rpool = ctx.enter_context(tc.tile_pool(name="rpool", bufs=2))
    opool = ctx.enter_context(tc.tile_pool(name="opool", bufs=2))
    # PSUM budget (8 banks): scores 2x[128,1024] (4) + U^T/denoms [128,1024]
    # (2) + filler pool 2x[128,512] (2). The filler pool decouples qkv/proj
    # background matmuls from the score/exp pipeline slots.
    psb = ctx.enter_context(tc.tile_pool(name="psb", bufs=2, space="PSUM"))
    psu = ctx.enter_context(tc.tile_pool(name="psu", bufs=1, space="PSUM"))
    pfill = ctx.enter_context(tc.tile_pool(name="pfill", bufs=2, space="PSUM"))

    # ---- persistent SBUF tensors, consolidated into big tiles so the whole
    # input load is 7 strided DMAs (SP DGE costs ~565ns per dma_start). The
    # order is deadline-driven: x token-half 0 + the f0/f4 feature chunks of
    # W_qk feed the first attention unit (~7us), then wv, then the rest.
    xT_all = wpool.tile([128, 8 * NT], BF, tag="xT", name="xT_all")
    wqk_all = wpool.tile([128, 8 * 1024], BF, tag="wqk", name="wqk_all")
    wv_all = wpool.tile([128, 8 * 512], BF, tag="wv", name="wv_all")
    wp_all = wpool.tile([128, 4 * 1024], BF, tag="wp", name="wp_all")

    xT_r = xT.rearrange("(a p) t -> p a t", p=128)  # [128, 8, 2048]
    xo_r = xT_all.rearrange("p (a t) -> p a t", a=8)
    wqk_r = wqk.rearrange("(a p) c -> p a c", p=128)  # [128, 8, 1024]
    wqo_r = wqk_all.rearrange("p (a c) -> p a c", a=8)
    for aa in (slice(0, 4), slice(4, 8)):
        nc.sync.dma_start(out=xo_r[:, aa, 0:1024], in_=xT_r[:, aa, 0:1024])
    # f0/f4 chunks: columns {0:128, 512:640} of each d-row block
    nc.sync.dma_start(out=wqo_r[:, :, 0:128], in_=wqk_r[:, :, 0:128])
    nc.sync.dma_start(out=wqo_r[:, :, 512:640], in_=wqk_r[:, :, 512:640])
    nc.sync.dma_start(
        out=wv_all.rearrange("p (a c) -> p a c", a=8),
        in_=wv.rearrange("(a p) c -> p a c", p=128),
    )
    nc.sync.dma_start(out=xo_r[:, :, 1024:2048], in_=xT_r[:, :, 1024:2048])
    nc.sync.dma_start(out=wqo_r[:, :, 128:512], in_=wqk_r[:, :, 128:512])
    nc.sync.dma_start(out=wqo_r[:, :, 640:1024], in_=wqk_r[:, :, 640:1024])
    nc.sync.dma_start(
        out=wp_all.rearrange("p (a c) -> p a c", a=4),
        in_=wp.rearrange("(a p) c -> p a c", p=128),
    )

    def xs(d, c0, c1):
        return xT_all[:, d * NT + c0 : d * NT + c1]

    def wqks(d, c0, c1):
        return wqk_all[:, d * 1024 + c0 : d * 1024 + c1]

    def wvs(d):
        return wv_all[:, d * 512 : (d + 1) * 512]

    def wps(c, c0, c1):
        return wp_all[:, c * 1024 + c0 : c * 1024 + c1]

    qkT = [qkpool.tile([128, NT], BF, tag=f"qkT{f}", name=f"qkT{f}") for f in range(8)]
    # V in per-head 65-wide blocks: col h*1040 + kt*65 + j; j==64 is the
    # constant-ones column that makes the PV matmul emit denominators.
    # whole-tile memset to 1.0: the v_unit drains overwrite columns j<64 of
    # each 65-wide block, leaving the j==64 ones-columns in place.
    v_all = vpool.tile([128, NH_LOC * 16 * 65], BF, tag="v", name="v_all")
    nc.vector.memset(v_all[:, :], 1.0)
    uhat = [upool.tile([128, NT], BF, tag=f"uh{p}", name=f"uh{p}") for p in range(4)]

    # ---- background units (run on the filler PSUM pool) -------------------
    def qk_sub(f, ts2):
        # qkT[f][:, ts2*512:(ts2+1)*512] = (x @ Wqk[:, f-chunk]).T slice
        ps = pfill.tile([128, 512], F32, tag="pf", name=f"qk_ps{f}_{ts2}")
        for d in range(8):
            nc.tensor.matmul(
                ps[:, :],
                wqks(d, f * 128, (f + 1) * 128),
                xs(d, ts2 * 512, (ts2 + 1) * 512),
                start=(d == 0),
                stop=(d == 7),
            )
        nc.vector.tensor_copy(out=qkT[f][:, ts2 * 512 : (ts2 + 1) * 512], in_=ps[:])

    def v_unit(t):
        ps = pfill.tile([128, 512], F32, tag="pf", name=f"v_ps{t}")
        for d in range(8):
            nc.tensor.matmul(
                ps[:, :],
                xs(d, t * 128, (t + 1) * 128),
                wvs(d),
                start=(d == 0),
                stop=(d == 7),
            )
        for hh in range(8):
            nc.vector.tensor_copy(
                out=v_all[:, hh * 1040 + t * 65 : hh * 1040 + t * 65 + 64],
                in_=ps[:, hh * 64 : (hh + 1) * 64],
            )

    def proj_sub(qt, es, pool=None, tag="pf"):
        # y[qt-tile, es-slice] partial over this core's 512 head dims
        pj = (pool or pfill).tile([128, 512], F32, tag=tag, name=f"pj{qt}_{es}")
        for c in range(4):
            nc.tensor.matmul(
                pj[:, :],
                uhat[c][:, qt * 128 : (qt + 1) * 128],
                wps(c, es * 512, (es + 1) * 512),
                start=(c == 0),
                stop=(c == 3),
            )
        ot = opool.tile([128, 512], F32, tag="out", name=f"ot{qt}_{es}")
        nc.vector.tensor_copy(out=ot, in_=pj[:])
        nc.sync.dma_start(
            out=y[qt * 128 : (qt + 1) * 128, es * 512 : (es + 1) * 512], in_=ot
        )

    # Two-stage tail projection: stage A (heads 0-5, c=0..2) weaves into the
    # last two attention units; stage B adds c=3 and drains. This shrinks the
    # post-attention serial tail from 16 full projections to 16 single
    # matmuls + fused adds.
    parts = {}

    def proj_a(qt, es):
        pj = pfill.tile([128, 512], F32, tag="pf", name=f"pa{qt}_{es}")
        for c in range(3):
            nc.tensor.matmul(
                pj[:, :],
                uhat[c][:, qt * 128 : (qt + 1) * 128],
                wps(c, es * 512, (es + 1) * 512),
                start=(c == 0),
                stop=(c == 2),
            )
        pt = partpool.tile([128, 512], BF, tag="part", name=f"part{qt}_{es}")
        nc.vector.tensor_copy(out=pt, in_=pj[:])
        parts[(qt, es)] = pt

    def proj_b(qt, es, pool=None, tag="pf"):
        pj = (pool or pfill).tile([128, 512], F32, tag=tag, name=f"pb{qt}_{es}")
        nc.tensor.matmul(
            pj[:, :],
            uhat[3][:, qt * 128 : (qt + 1) * 128],
            wps(3, es * 512, (es + 1) * 512),
            start=True,
            stop=True,
        )
        ot = opool.tile([128, 512], F32, tag="out", name=f"ot{qt}_{es}")
        nc.vector.scalar_tensor_tensor(
            out=ot,
            in0=pj[:],
            scalar=1.0,
            in1=parts[(qt, es)][:, :],
            op0=mybir.AluOpType.mult,
            op1=mybir.AluOpType.add,
        )
        nc.sync.dma_start(
            out=y[qt * 128 : (qt + 1) * 128, es * 512 : (es + 1) * 512], in_=ot
        )

    # ---- attention for one head, one query half ---------------------------
    # `fillers`: background units woven one-per-kt-step into this unit's
    # stream. Every filler MUST be emitted before the first instruction that
    # consumes its output (in-order engine queues deadlock otherwise); any
    # leftovers drain before the final PV matmuls.
    def attention_unit(h, half, fillers=()):
        fillers = list(fillers)
        f, r = h >> 1, (h & 1) * 64
        qh = qkT[f][r : r + 64, half * 1024 : (half + 1) * 1024]
        kh = qkT[4 + f][r : r + 64, :]
        vh = v_all[:, h * 1040 : (h + 1) * 1040]
        ut = psu.tile([128, 1024], F32, tag="ut", name=f"ut{h}_{half}")
        ebufs = []

        def pv(kt):
            eb = ebufs[kt // 4]
            for s in range(2):
                esl = eb[:, (kt % 4) * 1024 + s * 512 : (kt % 4) * 1024 + (s + 1) * 512]
                nc.tensor.matmul(
                    ut[0:65, s * 512 : (s + 1) * 512],
                    vh[:, kt * 65 : kt * 65 + 65],
                    esl,
                    start=(kt == 0),
                    stop=(kt == 15),
                )

        for kt in range(16):
            if kt % 4 == 0:
                ebufs.append(
                    epool.tile([128, 4096], BF, tag="e", name=f"e{h}_{half}_{kt // 4}")
                )
            # ready work (PV for kt-3, filler) goes BEFORE the QK score
            # groups: the in-order PE queue then reaches the QK slot-waits
            # with the previous exps already retired. Lag 3 (not 2) so each
            # v_unit filler lands strictly before the pv that consumes it.
            if kt >= 3:
                pv(kt - 3)
            if kt > 0 and fillers:
                item = fillers.pop(0)
                if item is not None:
                    item()
            st = psb.tile([128, 1024], F32, tag="st", name=f"st{h}_{half}_{kt}")
            for s in range(2):
                q0 = s * 512
                nc.tensor.matmul(
                    st[:, q0 : q0 + 512],
                    kh[:, kt * 128 : (kt + 1) * 128],
                    qh[:, q0 : q0 + 512],
                    start=True,
                    stop=True,
                )
            nc.scalar.activation(
                out=ebufs[-1][:, (kt % 4) * 1024 : (kt % 4 + 1) * 1024],
                in_=st[:],
                func=EXP,
                scale=SCALE,
            )
        while fillers:
            item = fillers.pop(0)
            if item is not None:
                item()
        pv(13)
        pv(14)
        pv(15)
        # Early PSUM drain on two engines in parallel: raw U^T (fp16, DVE)
        # and the denominator row (ScalarE copy). psu frees ~1.5us after the
        # last PV so the next unit's pv(0) never stalls on this unit's tail.
        # (both on DVE: putting drow on ScalarE delays the next unit's exps
        # behind it, which stalls the PE's score matmuls and resets the PE
        # pstate ramp)
        drow = spool.tile([1, 1024], F32, tag="drow", name=f"drow{h}_{half}", bufs=1)
        nc.vector.tensor_copy(out=drow[:, :], in_=ut[64:65, :])
        uraw = urawpool.tile([64, 1024], FP16, tag="uraw", name=f"uraw{h}_{half}")
        nc.vector.tensor_copy(out=uraw, in_=ut[0:64, :])
        # Off the critical path: reciprocal on a [128, 8] spread (free size 8,
        # ~200ns, vs 6.5us on a [1, 1024] row), then broadcast + normalize.
        rsp = spool.tile([128, 8], F32, tag="rsp", name=f"rsp{h}_{half}")
        nc.gpsimd.dma_start(
            out=rsp[:, :], in_=drow[0:1, :].rearrange("p (a b) -> p a b", a=128)
        )
        rspr = spool.tile([128, 8], F32, tag="rspr", name=f"rspr{h}_{half}")
        nc.vector.reciprocal(out=rspr[:, :], in_=rsp[:, :])
        rrow = spool.tile([1, 1024], F32, tag="rrow", name=f"rrow{h}_{half}", bufs=1)
        nc.gpsimd.dma_start(
            out=rrow[0:1, :].rearrange("p (a b) -> p a b", a=128), in_=rspr[:, :]
        )
        rb = rpool.tile([64, 1024], F32, tag="rb", name=f"rb{h}_{half}")
        nc.gpsimd.partition_broadcast(out_ap=rb[:, :], in_ap=rrow[0:1, :])
        nc.vector.tensor_mul(
            uhat[f][r : r + 64, half * 1024 : (half + 1) * 1024], uraw[:, :], rb[:, :]
        )
        if dbg is not None and h == 0 and half == 0:
            nc.sync.dma_start(out=dbg["drow"], in_=drow[0:1, :])
            nc.sync.dma_start(out=dbg["uraw"], in_=uraw[:, :])
            nc.sync.dma_start(out=dbg["rb"], in_=rb[0:1, :])

    # ---- schedule ---------------------------------------------------------
    def mk(fn, *args):
        return lambda: fn(*args)

    # lead-in: unit 0 needs Q half-0 of head 0 (f0 ts 0/1), the first key
    # quarter (f4 ts0) and v tile 0; the rest weaves in ahead of kt
    # deadlines (qk(4,ts) covers keys for kt in [4ts, 4ts+4); v(j) feeds
    # pv(j) from step j+2).
    qk_sub(0, 0)
    qk_sub(0, 1)
    qk_sub(4, 0)
    v_unit(0)
    v_unit(1)
    N_ = None
    unit_fillers = {
        # half 0: v tiles + remaining K quarters, then q/k features for the
        # following head pairs, then half-1 Q features. v(t) must be emitted
        # at kt step <= t+2 (pv lag 3); a unit's own qk(f_k, ts) at step
        # <= 4*ts - 1 (scores of step 4ts follow the filler in the step).
        # None entries space the real fillers out so every few steps give the
        # PE extra work - the in-order score matmuls otherwise catch up with
        # ScalarE's exp and stall ~100ns/step (resetting the PE pstate ramp).
        (0, 0): [mk(v_unit, 2), mk(v_unit, 3), mk(qk_sub, 4, 1), mk(v_unit, 4),
                 mk(v_unit, 5), mk(v_unit, 6), mk(qk_sub, 4, 2), mk(v_unit, 7),
                 mk(v_unit, 8), mk(v_unit, 9), mk(qk_sub, 4, 3), mk(v_unit, 10),
                 mk(v_unit, 11), mk(v_unit, 12), mk(v_unit, 13),
                 mk(v_unit, 14), mk(v_unit, 15)],
        (1, 0): [mk(qk_sub, 1, 0), N_, mk(qk_sub, 1, 1), N_,
                 mk(qk_sub, 5, 0), N_],
        (2, 0): [mk(qk_sub, 5, 1), mk(qk_sub, 2, 0), N_, mk(qk_sub, 5, 2),
                 mk(qk_sub, 2, 1), N_, N_, mk(qk_sub, 5, 3)],
        (3, 0): [mk(qk_sub, 6, 0), N_, mk(qk_sub, 3, 0), N_,
                 mk(qk_sub, 3, 1), N_],
        (4, 0): [mk(qk_sub, 6, 1), N_, N_, mk(qk_sub, 6, 2), N_, N_,
                 mk(qk_sub, 6, 3), N_, N_, mk(qk_sub, 7, 0), N_, N_],
        (5, 0): [mk(qk_sub, 0, 2), N_, mk(qk_sub, 0, 3), N_,
                 mk(qk_sub, 1, 2), N_, mk(qk_sub, 1, 3), N_],
        (6, 0): [mk(qk_sub, 7, 1), mk(qk_sub, 2, 2), N_, N_,
                 mk(qk_sub, 7, 2), mk(qk_sub, 2, 3), N_, N_,
                 mk(qk_sub, 7, 3), mk(qk_sub, 3, 2), N_, N_],
        (7, 0): [mk(qk_sub, 3, 3)],
        # half 1: weave the half-0 output projection (ready once all half-0
        # units finished), then stage-A partials of the half-1 projection
        # into the last two units (they need heads 0-5 = units through (5,1)).
        (0, 1): [N_, mk(proj_sub, 0, 0), N_, N_, mk(proj_sub, 0, 1), N_],
        (1, 1): [mk(proj_sub, 1, 0), N_, N_, mk(proj_sub, 1, 1), N_, N_,
                 mk(proj_sub, 2, 0), N_],
        (2, 1): [mk(proj_sub, 2, 1), N_, N_, mk(proj_sub, 3, 0), N_, N_,
                 mk(proj_sub, 3, 1), N_],
        (3, 1): [mk(proj_sub, 4, 0), N_, N_, mk(proj_sub, 4, 1), N_, N_,
                 mk(proj_sub, 5, 0), N_],
        (4, 1): [mk(proj_sub, 5, 1), N_, N_, mk(proj_sub, 6, 0), N_, N_,
                 mk(proj_sub, 6, 1), N_],
        (5, 1): [mk(proj_sub, 7, 0), N_, N_, mk(proj_sub, 7, 1)],
        (6, 1): [N_, mk(proj_a, 8, 0), N_, mk(proj_a, 8, 1), N_,
                 mk(proj_a, 9, 0), N_, mk(proj_a, 9, 1), N_, mk(proj_a, 10, 0),
                 N_, mk(proj_a, 10, 1), N_, mk(proj_a, 11, 0), mk(proj_a, 11, 1)],
        (7, 1): [mk(proj_a, 12, 0), N_, mk(proj_a, 12, 1), N_,
                 mk(proj_a, 13, 0), N_, mk(proj_a, 13, 1), N_, mk(proj_a, 14, 0),
                 N_, mk(proj_a, 14, 1), N_, mk(proj_a, 15, 0), N_,
                 mk(proj_a, 15, 1)],
    }
    for half in range(2):
        for h in range(8):
            attention_unit(h, half, unit_fillers[(h, half)])
    for qt in range(8, 16):
        for es in range(2):
            if (qt * 2 + es) % 2 == 0:
                proj_b(qt, es)
            else:
                proj_b(qt, es, pool=psb, tag="st")
    if dbg is not None:
        nc.sync.dma_start(out=dbg["qk0"], in_=qkT[0][:, :])
        nc.sync.dma_start(out=dbg["qk4"], in_=qkT[4][:, :])
        nc.sync.dma_start(out=dbg["vv"], in_=v_all[:, :])
        nc.sync.dma_start(out=dbg["uh0"], in_=uhat[0][:, :])


_NC_CACHE = {}


def _build_nc():
    if "nc" in _NC_CACHE:
        return _NC_CACHE["nc"]
    nc = bacc.Bacc("TRN2", target_bir_lowering=False, debug=False, num_devices=N_CORES)
    xT = nc.dram_tensor("xT", [D, NT], BF, kind="ExternalInput").ap()
    wqk = nc.dram_tensor("wqk", [D, 1024], BF, kind="ExternalInput").ap()
    wv = nc.dram_tensor("wv", [D, 512], BF, kind="ExternalInput").ap()
    wp = nc.dram_tensor("wp", [512, 1024], BF, kind="ExternalInput").ap()
    y = nc.dram_tensor("y", [NT, 1024], F32, kind="ExternalOutput").ap()
    dbg = None
    if DEBUG_DUMP:
        dbg = {
            "drow": nc.dram_tensor("dbg_drow", [1, 1024], F32, kind="ExternalOutput").ap(),
            "uraw": nc.dram_tensor("dbg_uraw", [64, 1024], FP16, kind="ExternalOutput").ap(),
            "rb": nc.dram_tensor("dbg_rb", [1, 1024], F32, kind="ExternalOutput").ap(),
            "qk0": nc.dram_tensor("dbg_qk0", [128, NT], BF, kind="ExternalOutput").ap(),
            "qk4": nc.dram_tensor("dbg_qk4", [128, NT], BF, kind="ExternalOutput").ap(),
            "vv": nc.dram_tensor("dbg_vv", [128, 8320], BF, kind="ExternalOutput").ap(),
            "uh0": nc.dram_tensor("dbg_uh0", [128, NT], BF, kind="ExternalOutput").ap(),
        }
    from contextlib import ExitStack

    with tile.TileContext(nc) as tc, ExitStack() as ctx:
        _body(tc, ctx, y, xT, wqk, wv, wp, dbg=dbg)
    nc.compile()
    _NC_CACHE["nc"] = nc
    return nc


def _prepare_in_maps(x, W_qkv, W_proj):
    x = np.asarray(x, dtype=np.float32)
    W_qkv = np.asarray(W_qkv, dtype=np.float32)
    W_proj = np.asarray(W_proj, dtype=np.float32)
    in_maps = []
    for c in range(N_CORES):
        b, hg = divmod(c, 2)
        cs = slice(hg * 512, (hg + 1) * 512)
        xTc = np.ascontiguousarray(x[b].T).astype(BF16)
        wqk_c = np.ascontiguousarray(
            np.concatenate([W_qkv[:, 0:1024][:, cs], W_qkv[:, 1024:2048][:, cs]], axis=1)
        ).astype(BF16)
        wv_c = np.ascontiguousarray(W_qkv[:, 2048:3072][:, cs]).astype(BF16)
        wp_c = np.ascontiguousarray(W_proj[cs, :]).astype(BF16)
        in_maps.append({"xT": xTc, "wqk": wqk_c, "wv": wv_c, "wp": wp_c})
    return in_maps


def _run(x, W_qkv, W_proj, b_proj, trace=False):
    nc = _build_nc()
    in_maps = _prepare_in_maps(x, W_qkv, W_proj)
    res = bass_utils.run_bass_kernel_spmd(
        nc, in_maps, core_ids=list(range(N_CORES)), trace=trace
    )
    b_proj = np.asarray(b_proj, dtype=np.float32)
    y = np.empty((4, NT, D), dtype=np.float32)
    for b in range(4):
        y[b] = res.results[2 * b]["y"] + res.results[2 * b + 1]["y"] + b_proj[None, :]
    return y, res


def kernel(x, W_qkv, W_proj, b_proj):
    y, _ = _run(x, W_qkv, W_proj, b_proj, trace=False)
    return y


# revision 37
# speedup vs baseline: 1.1920x; 1.1920x over previous
"""Trainium2 Bass kernel for nn_Attention (B=4, N=2048, D=1024, H=16, Hd=64).

Sharding: 8 cores = 4 batches x 2 head-groups. Core c handles batch c//2 and
heads [ (c%2)*8, (c%2)*8+8 ).  Each core computes qkv projections for its
heads, attention, and a partial output projection (contraction over its 512
head-dims of W_proj). Host sums the two partials per batch and adds b_proj.

Per-core kernel (all matmuls bf16 with fp32 PSUM accumulation). Work is
organized as 16 attention units (8 heads x 2 query halves of 1024):
  - qkT[f][p, t]: q/k features on partitions (f 0-3 = Q of head pairs,
    4-7 = K), produced by background qk_sub units.
  - v_all[k, h*1040 + kt*65 + j]: V in keys-major per-head 65-wide blocks
    whose 65th column is constant 1.0. The PV matmul (lhsT = V block,
    M=65) then accumulates BOTH U^T (rows 0-63) and the softmax
    denominators sum_k E[k,q] (row 64) in one PSUM tile - no separate
    E-accumulation pass is needed.
  - per unit: 16 key tiles: S^T = K^T Q (keys on partitions), E = exp
    (ScalarE, bf16), U^T/denoms += V_ext^T E (PSUM, lag-2 behind exp).
    Tail: recip(denoms) -> gpsimd partition_broadcast -> normalize into
    uhat (bf16).
  - proj: y[q, e] partial = sum_hd uhat Wp, background-woven; host adds
    the pair partials + bias.
"""

import os
import sys
import types

import numpy as np

# --- environment bootstrap (grading env == dev env: axon-tunneled trn2) ----
for _p in ("/opt/trn_rl_repo", "/root/.axon_site/_ro/trn_rl_repo"):
    if _p not in sys.path and os.path.isdir(_p):
        sys.path.append(_p)

import ml_dtypes  # noqa: E402

BF16 = ml_dtypes.bfloat16


def _install_ntff_shim():
    """antenv.axon_hooks is missing on this image; provide it and register the
    ctypes NTFF hook so trace=True can report HW exec time."""
    if "antenv.axon_hooks" in sys.modules:
        return
    mod = types.ModuleType("antenv.axon_hooks")
    mod._hook = None
    mod.set_axon_ntff_profile_hook = lambda h: setattr(mod, "_hook", h)
    mod.get_axon_ntff_profile_hook = lambda: mod._hook
    sys.modules["antenv.axon_hooks"] = mod
    try:
        import antenv

        antenv.axon_hooks = mod
    except ImportError:
        pass
    try:
        from trn_agent_boot.trn_boot import _ntff_profile_via_ctypes

        hook = _ntff_profile_via_ctypes("/opt/axon/libaxon_pjrt.so")
        if hook is not None:
            mod.set_axon_ntff_profile_hook(hook)
    except Exception:
        pass


_install_ntff_shim()

import concourse.bacc as bacc  # noqa: E402
import concourse.bass as bass  # noqa: E402
import concourse.tile as tile  # noqa: E402
import concourse.bass_isa as bass_isa  # noqa: E402
from concourse import mybir  # noqa: E402
import concourse.bass_utils as bass_utils  # noqa: E402

# no S3 in the container; keep NTFF artifacts local
bass_utils.upload_artifacts = lambda tmpdir: tmpdir

F32 = mybir.dt.float32
BF = mybir.dt.bfloat16
FP16 = mybir.dt.float16
EXP = mybir.ActivationFunctionType.Exp
CPY = mybir.ActivationFunctionType.Copy

N_CORES = 8
NT = 2048  # tokens
D = 1024  # d_model
NH_LOC = 8  # heads per core
HD = 64  # head dim
SCALE = HD**-0.5


DEBUG_DUMP = bool(int(os.environ.get("KBASS_DEBUG", "0")))


def _body(tc: "tile.TileContext", ctx, y, xT, wqk, wv, wp, dbg=None):
    nc = tc.nc

    wpool = ctx.enter_context(tc.tile_pool(name="wpool", bufs=1))
    qkpool = ctx.enter_context(tc.tile_pool(name="qkpool", bufs=1))
    vpool = ctx.enter_context(tc.tile_pool(name="vpool", bufs=1))
    upool = ctx.enter_context(tc.tile_pool(name="upool", bufs=1))
    urawpool = ctx.enter_context(tc.tile_pool(name="urawpool", bufs=2))
    partpool = ctx.enter_context(tc.tile_pool(name="partpool", bufs=16))
    epool = ctx.enter_context(tc.tile_pool(name="epool", bufs=3))
    spool = ctx.enter_context(tc.tile_pool(name="spool", bufs=2))
    rpool = ctx.enter_context(tc.tile_pool(name="rpool", bufs=1))
    opool = ctx.enter_context(tc.tile_pool(name="opool", bufs=2))
    # PSUM budget (8 banks): scores 2x[128,1024] (4) + U^T/denoms [128,1024]
    # (2) + filler pool 2x[128,512] (2). The filler pool decouples qkv/proj
    # background matmuls from the score/exp pipeline slots.
    psb = ctx.enter_context(tc.tile_pool(name="psb", bufs=2, space="PSUM"))
    psu = ctx.enter_context(tc.tile_pool(name="psu", bufs=1, space="PSUM"))
    pfill = ctx.enter_context(tc.tile_pool(name="pfill", bufs=2, space="PSUM"))

    # ---- persistent SBUF tensors, consolidated into big tiles so the whole
    # input load is 7 strided DMAs (SP DGE costs ~565ns per dma_start). The
    # order is deadline-driven: x token-half 0 + the f0/f4 feature chunks of
    # W_qk feed the first attention unit (~7us), then wv, then the rest.
    xT_all = wpool.tile([128, 8 * NT], BF, tag="xT", name="xT_all")
    wqk_all = wpool.tile([128, 8 * 1024], BF, tag="wqk", name="wqk_all")
    wv_all = wpool.tile([128, 8 * 512], BF, tag="wv", name="wv_all")
    wp_all = wpool.tile([128, 4 * 1024], BF, tag="wp", name="wp_all")

    xT_r = xT.rearrange("(a p) t -> p a t", p=128)  # [128, 8, 2048]
    xo_r = xT_all.rearrange("p (a t) -> p a t", a=8)
    wqk_r = wqk.rearrange("(a p) c -> p a c", p=128)  # [128, 8, 1024]
    wqo_r = wqk_all.rearrange("p (a c) -> p a c", a=8)
    for aa in (slice(0, 4), slice(4, 8)):
        nc.sync.dma_start(out=xo_r[:, aa, 0:1024], in_=xT_r[:, aa, 0:1024])
    # f0/f4 chunks: columns {0:128, 512:640} of each d-row block
    nc.sync.dma_start(out=wqo_r[:, :, 0:128], in_=wqk_r[:, :, 0:128])
    nc.sync.dma_start(out=wqo_r[:, :, 512:640], in_=wqk_r[:, :, 512:640])
    nc.sync.dma_start(
        out=wv_all.rearrange("p (a c) -> p a c", a=8),
        in_=wv.rearrange("(a p) c -> p a c", p=128),
    )
    nc.sync.dma_start(out=xo_r[:, :, 1024:2048], in_=xT_r[:, :, 1024:2048])
    nc.sync.dma_start(out=wqo_r[:, :, 128:512], in_=wqk_r[:, :, 128:512])
    nc.sync.dma_start(out=wqo_r[:, :, 640:1024], in_=wqk_r[:, :, 640:1024])
    nc.sync.dma_start(
        out=wp_all.rearrange("p (a c) -> p a c", a=4),
        in_=wp.rearrange("(a p) c -> p a c", p=128),
    )

    def xs(d, c0, c1):
        return xT_all[:, d * NT + c0 : d * NT + c1]

    def wqks(d, c0, c1):
        return wqk_all[:, d * 1024 + c0 : d * 1024 + c1]

    def wvs(d):
        return wv_all[:, d * 512 : (d + 1) * 512]

    def wps(c, c0, c1):
        return wp_all[:, c * 1024 + c0 : c * 1024 + c1]

    qkT = [qkpool.tile([128, NT], BF, tag=f"qkT{f}", name=f"qkT{f}") for f in range(8)]
    # V in per-head 65-wide blocks: col h*1040 + kt*65 + j; j==64 is the
    # constant-ones column that makes the PV matmul emit denominators.
    # whole-tile memset to 1.0: the v_unit drains overwrite columns j<64 of
    # each 65-wide block, leaving the j==64 ones-columns in place.
    v_all = vpool.tile([128, NH_LOC * 16 * 65], BF, tag="v", name="v_all")
    nc.vector.memset(v_all[:, :], 1.0)
    uhat = [upool.tile([128, NT], BF, tag=f"uh{p}", name=f"uh{p}") for p in range(4)]

    # ---- background units (run on the filler PSUM pool) -------------------
    def qk_sub(f, ts2):
        # qkT[f][:, ts2*512:(ts2+1)*512] = (x @ Wqk[:, f-chunk]).T slice
        ps = pfill.tile([128, 512], F32, tag="pf", name=f"qk_ps{f}_{ts2}")
        for d in range(8):
            nc.tensor.matmul(
                ps[:, :],
                wqks(d, f * 128, (f + 1) * 128),
                xs(d, ts2 * 512, (ts2 + 1) * 512),
                start=(d == 0),
                stop=(d == 7),
            )
        nc.vector.tensor_copy(out=qkT[f][:, ts2 * 512 : (ts2 + 1) * 512], in_=ps[:])

    def v_unit(t):
        ps = pfill.tile([128, 512], F32, tag="pf", name=f"v_ps{t}")
        for d in range(8):
            nc.tensor.matmul(
                ps[:, :],
                xs(d, t * 128, (t + 1) * 128),
                wvs(d),
                start=(d == 0),
                stop=(d == 7),
            )
        for hh in range(8):
            nc.vector.tensor_copy(
                out=v_all[:, hh * 1040 + t * 65 : hh * 1040 + t * 65 + 64],
                in_=ps[:, hh * 64 : (hh + 1) * 64],
            )

    def proj_sub(qt, es, pool=None, tag="pf"):
        # y[qt-tile, es-slice] partial over this core's 512 head dims
        pj = (pool or pfill).tile([128, 512], F32, tag=tag, name=f"pj{qt}_{es}")
        for c in range(4):
            nc.tensor.matmul(
                pj[:, :],
                uhat[c][:, qt * 128 : (qt + 1) * 128],
                wps(c, es * 512, (es + 1) * 512),
                start=(c == 0),
                stop=(c == 3),
            )
        ot = opool.tile([128, 512], F32, tag="out", name=f"ot{qt}_{es}")
        nc.vector.tensor_copy(out=ot, in_=pj[:])
        nc.sync.dma_start(
            out=y[qt * 128 : (qt + 1) * 128, es * 512 : (es + 1) * 512], in_=ot
        )

    # Two-stage tail projection: stage A (heads 0-5, c=0..2) weaves into the
    # last two attention units; stage B adds c=3 and drains. This shrinks the
    # post-attention serial tail from 16 full projections to 16 single
    # matmuls + fused adds.
    parts = {}

    def proj_a(qt, es):
        pj = pfill.tile([128, 512], F32, tag="pf", name=f"pa{qt}_{es}")
        for c in range(3):
            nc.tensor.matmul(
                pj[:, :],
                uhat[c][:, qt * 128 : (qt + 1) * 128],
                wps(c, es * 512, (es + 1) * 512),
                start=(c == 0),
                stop=(c == 2),
            )
        pt = partpool.tile([128, 512], BF, tag="part", name=f"part{qt}_{es}")
        nc.vector.tensor_copy(out=pt, in_=pj[:])
        parts[(qt, es)] = pt

    def proj_b(qt, es, pool=None, tag="pf"):
        pj = (pool or pfill).tile([128, 512], F32, tag=tag, name=f"pb{qt}_{es}")
        nc.tensor.matmul(
            pj[:, :],
            uhat[3][:, qt * 128 : (qt + 1) * 128],
            wps(3, es * 512, (es + 1) * 512),
            start=True,
            stop=True,
        )
        ot = opool.tile([128, 512], F32, tag="out", name=f"ot{qt}_{es}")
        nc.vector.scalar_tensor_tensor(
            out=ot,
            in0=pj[:],
            scalar=1.0,
            in1=parts[(qt, es)][:, :],
            op0=mybir.AluOpType.mult,
            op1=mybir.AluOpType.add,
        )
        nc.sync.dma_start(
            out=y[qt * 128 : (qt + 1) * 128, es * 512 : (es + 1) * 512], in_=ot
        )

    # ---- attention for one head, one query half ---------------------------
    # `fillers`: background units woven one-per-kt-step into this unit's
    # stream. Every filler MUST be emitted before the first instruction that
    # consumes its output (in-order engine queues deadlock otherwise); any
    # leftovers drain before the final PV matmuls.
    def attention_unit(h, half, fillers=()):
        fillers = list(fillers)
        f, r = h >> 1, (h & 1) * 64
        qh = qkT[f][r : r + 64, half * 1024 : (half + 1) * 1024]
        kh = qkT[4 + f][r : r + 64, :]
        vh = v_all[:, h * 1040 : (h + 1) * 1040]
        ut = psu.tile([128, 1024], F32, tag="ut", name=f"ut{h}_{half}")
        ebufs = []

        def pv(kt):
            eb = ebufs[kt // 4]
            for s in range(2):
                esl = eb[:, (kt % 4) * 1024 + s * 512 : (kt % 4) * 1024 + (s + 1) * 512]
                nc.tensor.matmul(
                    ut[0:65, s * 512 : (s + 1) * 512],
                    vh[:, kt * 65 : kt * 65 + 65],
                    esl,
                    start=(kt == 0),
                    stop=(kt == 15),
                )

        for kt in range(16):
            if kt % 4 == 0:
                ebufs.append(
                    epool.tile([128, 4096], BF, tag="e", name=f"e{h}_{half}_{kt // 4}")
                )
            # ready work (PV for kt-3, filler) goes BEFORE the QK score
            # groups: the in-order PE queue then reaches the QK slot-waits
            # with the previous exps already retired. Lag 3 (not 2) so each
            # v_unit filler lands strictly before the pv that consumes it.
            if kt >= 3:
                pv(kt - 3)
            if kt > 0 and fillers:
                item = fillers.pop(0)
                if item is not None:
                    item()
            st = psb.tile([128, 1024], F32, tag="st", name=f"st{h}_{half}_{kt}")
            for s in range(2):
                q0 = s * 512
                nc.tensor.matmul(
                    st[:, q0 : q0 + 512],
                    kh[:, kt * 128 : (kt + 1) * 128],
                    qh[:, q0 : q0 + 512],
                    start=True,
                    stop=True,
                )
            nc.scalar.activation(
                out=ebufs[-1][:, (kt % 4) * 1024 : (kt % 4 + 1) * 1024],
                in_=st[:],
                func=EXP,
                scale=SCALE,
            )
        while fillers:
            item = fillers.pop(0)
            if item is not None:
                item()
        pv(13)
        pv(14)
        pv(15)
        if h == 7 and half == 1:
            # Split tail for the final unit: process the denominators in two
            # 512-col chunks so uhat[3]'s first half unlocks the stage-B
            # projections ~2us earlier (nothing else overlaps this chain).
            drows = []
            for cc in range(2):
                dr = spool.tile(
                    [1, 512], F32, tag="drow", name=f"drc{cc}", bufs=1
                )
                nc.vector.tensor_copy(
                    out=dr[:, :], in_=ut[64:65, cc * 512 : (cc + 1) * 512]
                )
                drows.append(dr)
            uraw = urawpool.tile([64, 1024], FP16, tag="uraw", name="uraw71")
            for cc in range(2):
                rsp = spool.tile([128, 4], F32, tag="rsp", name=f"rsc{cc}")
                nc.gpsimd.dma_start(
                    out=rsp[:, :],
                    in_=drows[cc][0:1, :].rearrange("p (a b) -> p a b", a=128),
                )
                rspr = spool.tile([128, 4], F32, tag="rspr", name=f"rrc{cc}")
                nc.vector.reciprocal(out=rspr[:, :], in_=rsp[:, :])
                rrow = spool.tile(
                    [1, 512], F32, tag="rrow", name=f"rwc{cc}", bufs=1
                )
                nc.gpsimd.dma_start(
                    out=rrow[0:1, :].rearrange("p (a b) -> p a b", a=128),
                    in_=rspr[:, :],
                )
                rb = rpool.tile([64, 512], F32, tag="rb", name=f"rbc{cc}")
                nc.gpsimd.partition_broadcast(out_ap=rb[:, :], in_ap=rrow[0:1, :])
                if cc == 0:
                    nc.vector.tensor_copy(out=uraw, in_=ut[0:64, :])
                nc.vector.tensor_mul(
                    uhat[f][
                        r : r + 64,
                        half * 1024 + cc * 512 : half * 1024 + (cc + 1) * 512,
                    ],
                    uraw[:, cc * 512 : (cc + 1) * 512],
                    rb[:, :],
                )
            return
        # Early PSUM drain on two engines in parallel: raw U^T (fp16, DVE)
        # and the denominator row (ScalarE copy). psu frees ~1.5us after the
        # last PV so the next unit's pv(0) never stalls on this unit's tail.
        # (both on DVE: putting drow on ScalarE delays the next unit's exps
        # behind it, which stalls the PE's score matmuls and resets the PE
        # pstate ramp)
        drow = spool.tile([1, 1024], F32, tag="drow", name=f"drow{h}_{half}", bufs=1)
        nc.vector.tensor_copy(out=drow[:, :], in_=ut[64:65, :])
        uraw = urawpool.tile([64, 1024], FP16, tag="uraw", name=f"uraw{h}_{half}")
        nc.vector.tensor_copy(out=uraw, in_=ut[0:64, :])
        # Off the critical path: reciprocal on a [128, 8] spread (free size 8,
        # ~200ns, vs 6.5us on a [1, 1024] row), then broadcast + normalize.
        rsp = spool.tile([128, 8], F32, tag="rsp", name=f"rsp{h}_{half}")
        nc.gpsimd.dma_start(
            out=rsp[:, :], in_=drow[0:1, :].rearrange("p (a b) -> p a b", a=128)
        )
        rspr = spool.tile([128, 8], F32, tag="rspr", name=f"rspr{h}_{half}")
        nc.vector.reciprocal(out=rspr[:, :], in_=rsp[:, :])
        rrow = spool.tile([1, 1024], F32, tag="rrow", name=f"rrow{h}_{half}", bufs=1)
        nc.gpsimd.dma_start(
            out=rrow[0:1, :].rearrange("p (a b) -> p a b", a=128), in_=rspr[:, :]
        )
        rb = rpool.tile([64, 1024], F32, tag="rb", name=f"rb{h}_{half}")
        nc.gpsimd.partition_broadcast(out_ap=rb[:, :], in_ap=rrow[0:1, :])
        nc.vector.tensor_mul(
            uhat[f][r : r + 64, half * 1024 : (half + 1) * 1024], uraw[:, :], rb[:, :]
        )
        if dbg is not None and h == 0 and half == 0:
            nc.sync.dma_start(out=dbg["drow"], in_=drow[0:1, :])
            nc.sync.dma_start(out=dbg["uraw"], in_=uraw[:, :])
            nc.sync.dma_start(out=dbg["rb"], in_=rb[0:1, :])

    # ---- schedule ---------------------------------------------------------
    def mk(fn, *args):
        return lambda: fn(*args)

    # lead-in: unit 0 needs Q half-0 of head 0 (f0 ts 0/1), the first key
    # quarter (f4 ts0) and v tile 0; the rest weaves in ahead of kt
    # deadlines (qk(4,ts) covers keys for kt in [4ts, 4ts+4); v(j) feeds
    # pv(j) from step j+2).
    qk_sub(0, 0)
    qk_sub(0, 1)
    qk_sub(4, 0)
    v_unit(0)
    v_unit(1)
    N_ = None
    unit_fillers = {
        # half 0: v tiles + remaining K quarters, then q/k features for the
        # following head pairs, then half-1 Q features. v(t) must be emitted
        # at kt step <= t+2 (pv lag 3); a unit's own qk(f_k, ts) at step
        # <= 4*ts - 1 (scores of step 4ts follow the filler in the step).
        # None entries space the real fillers out so every few steps give the
        # PE extra work - the in-order score matmuls otherwise catch up with
        # ScalarE's exp and stall ~100ns/step (resetting the PE pstate ramp).
        (0, 0): [mk(v_unit, 2), mk(v_unit, 3), mk(qk_sub, 4, 1), mk(v_unit, 4),
                 mk(v_unit, 5), mk(v_unit, 6), mk(qk_sub, 4, 2), mk(v_unit, 7),
                 mk(v_unit, 8), mk(v_unit, 9), mk(qk_sub, 4, 3), mk(v_unit, 10),
                 mk(v_unit, 11), mk(v_unit, 12), mk(v_unit, 13),
                 mk(v_unit, 14), mk(v_unit, 15)],
        (1, 0): [mk(qk_sub, 1, 0), N_, mk(qk_sub, 1, 1), N_,
                 mk(qk_sub, 5, 0), N_],
        (2, 0): [mk(qk_sub, 5, 1), mk(qk_sub, 2, 0), N_, mk(qk_sub, 5, 2),
                 mk(qk_sub, 2, 1), N_, N_, mk(qk_sub, 5, 3)],
        (3, 0): [mk(qk_sub, 6, 0), N_, mk(qk_sub, 3, 0), N_,
                 mk(qk_sub, 3, 1), N_],
        (4, 0): [mk(qk_sub, 6, 1), N_, N_, mk(qk_sub, 6, 2), N_, N_,
                 mk(qk_sub, 6, 3), N_, N_, mk(qk_sub, 7, 0), N_, N_],
        (5, 0): [mk(qk_sub, 0, 2), N_, mk(qk_sub, 0, 3), N_,
                 mk(qk_sub, 1, 2), N_, mk(qk_sub, 1, 3), N_],
        (6, 0): [mk(qk_sub, 7, 1), mk(qk_sub, 2, 2), N_, N_,
                 mk(qk_sub, 7, 2), mk(qk_sub, 2, 3), N_, N_,
                 mk(qk_sub, 7, 3), mk(qk_sub, 3, 2), N_, N_],
        (7, 0): [mk(qk_sub, 3, 3)],
        # half 1: weave the half-0 output projection (ready once all half-0
        # units finished), then stage-A partials of the half-1 projection
        # into the last two units (they need heads 0-5 = units through (5,1)).
        (0, 1): [N_, mk(proj_sub, 0, 0), N_, N_, mk(proj_sub, 0, 1), N_],
        (1, 1): [mk(proj_sub, 1, 0), N_, N_, mk(proj_sub, 1, 1), N_, N_,
                 mk(proj_sub, 2, 0), N_],
        (2, 1): [mk(proj_sub, 2, 1), N_, N_, mk(proj_sub, 3, 0), N_, N_,
                 mk(proj_sub, 3, 1), N_],
        (3, 1): [mk(proj_sub, 4, 0), N_, N_, mk(proj_sub, 4, 1), N_, N_,
                 mk(proj_sub, 5, 0), N_],
        (4, 1): [mk(proj_sub, 5, 1), N_, N_, mk(proj_sub, 6, 0), N_, N_,
                 mk(proj_sub, 6, 1), N_],
        (5, 1): [mk(proj_sub, 7, 0), N_, N_, mk(proj_sub, 7, 1)],
        (6, 1): [N_, mk(proj_a, 8, 0), N_, mk(proj_a, 8, 1), N_,
                 mk(proj_a, 9, 0), N_, mk(proj_a, 9, 1), N_, mk(proj_a, 10, 0),
                 N_, mk(proj_a, 10, 1), N_, mk(proj_a, 11, 0), mk(proj_a, 11, 1)],
        (7, 1): [mk(proj_a, 12, 0), N_, mk(proj_a, 12, 1), N_,
                 mk(proj_a, 13, 0), N_, mk(proj_a, 13, 1), N_, mk(proj_a, 14, 0),
                 N_, mk(proj_a, 14, 1), N_, mk(proj_a, 15, 0), N_,
                 mk(proj_a, 15, 1)],
    }
    for half in range(2):
        for h in range(8):
            attention_unit(h, half, unit_fillers[(h, half)])
    for qt in range(8, 16):
        for es in range(2):
            if (qt * 2 + es) % 2 == 0:
                proj_b(qt, es)
            else:
                proj_b(qt, es, pool=psb, tag="st")
    if dbg is not None:
        nc.sync.dma_start(out=dbg["qk0"], in_=qkT[0][:, :])
        nc.sync.dma_start(out=dbg["qk4"], in_=qkT[4][:, :])
        nc.sync.dma_start(out=dbg["vv"], in_=v_all[:, :])
        nc.sync.dma_start(out=dbg["uh0"], in_=uhat[0][:, :])


_NC_CACHE = {}


def _build_nc():
    if "nc" in _NC_CACHE:
        return _NC_CACHE["nc"]
    nc = bacc.Bacc("TRN2", target_bir_lowering=False, debug=False, num_devices=N_CORES)
    xT = nc.dram_tensor("xT", [D, NT], BF, kind="ExternalInput").ap()
    wqk = nc.dram_tensor("wqk", [D, 1024], BF, kind="ExternalInput").ap()
    wv = nc.dram_tensor("wv", [D, 512], BF, kind="ExternalInput").ap()
    wp = nc.dram_tensor("wp", [512, 1024], BF, kind="ExternalInput").ap()
    y = nc.dram_tensor("y", [NT, 1024], F32, kind="ExternalOutput").ap()
    dbg = None
    if DEBUG_DUMP:
        dbg = {
            "drow": nc.dram_tensor("dbg_drow", [1, 1024], F32, kind="ExternalOutput").ap(),
            "uraw": nc.dram_tensor("dbg_uraw", [64, 1024], FP16, kind="ExternalOutput").ap(),
            "rb": nc.dram_tensor("dbg_rb", [1, 1024], F32, kind="ExternalOutput").ap(),
            "qk0": nc.dram_tensor("dbg_qk0", [128, NT], BF, kind="ExternalOutput").ap(),
            "qk4": nc.dram_tensor("dbg_qk4", [128, NT], BF, kind="ExternalOutput").ap(),
            "vv": nc.dram_tensor("dbg_vv", [128, 8320], BF, kind="ExternalOutput").ap(),
            "uh0": nc.dram_tensor("dbg_uh0", [128, NT], BF, kind="ExternalOutput").ap(),
        }
    from contextlib import ExitStack

    with tile.TileContext(nc) as tc, ExitStack() as ctx:
        _body(tc, ctx, y, xT, wqk, wv, wp, dbg=dbg)
    nc.compile()
    _NC_CACHE["nc"] = nc
    return nc


def _prepare_in_maps(x, W_qkv, W_proj):
    x = np.asarray(x, dtype=np.float32)
    W_qkv = np.asarray(W_qkv, dtype=np.float32)
    W_proj = np.asarray(W_proj, dtype=np.float32)
    in_maps = []
    for c in range(N_CORES):
        b, hg = divmod(c, 2)
        cs = slice(hg * 512, (hg + 1) * 512)
        xTc = np.ascontiguousarray(x[b].T).astype(BF16)
        wqk_c = np.ascontiguousarray(
            np.concatenate([W_qkv[:, 0:1024][:, cs], W_qkv[:, 1024:2048][:, cs]], axis=1)
        ).astype(BF16)
        wv_c = np.ascontiguousarray(W_qkv[:, 2048:3072][:, cs]).astype(BF16)
        wp_c = np.ascontiguousarray(W_proj[cs, :]).astype(BF16)
        in_maps.append({"xT": xTc, "wqk": wqk_c, "wv": wv_c, "wp": wp_c})
    return in_maps


def _run(x, W_qkv, W_proj, b_proj, trace=False):
    nc = _build_nc()
    in_maps = _prepare_in_maps(x, W_qkv, W_proj)
    res = bass_utils.run_bass_kernel_spmd(
        nc, in_maps, core_ids=list(range(N_CORES)), trace=trace
    )
    b_proj = np.asarray(b_proj, dtype=np.float32)
    y = np.empty((4, NT, D), dtype=np.float32)
    for b in range(4):
        y[b] = res.results[2 * b]["y"] + res.results[2 * b + 1]["y"] + b_proj[None, :]
    return y, res


def kernel(x, W_qkv, W_proj, b_proj):
    y, _ = _run(x, W_qkv, W_proj, b_proj, trace=False)
    return y
